# revision 1
# baseline (speedup 1.0000x reference)
"""Trainium2 Bass kernel for the FCNN color-counter valuation function.

Computes out[i] = a[i, int(z[i, attr_index])] * 0.999 for i in [0, B).

Strategy: pure data parallel over 8 NeuronCores (batch sharded). Per core,
rows are laid out partition-major ([128 partitions, J rows each]) so every
DMA is a large contiguous-per-partition transfer. The gather is computed as
a one-hot dot product on the vector engine:
    mask = (z[:, attr] == iota_c)        # broadcast compare, [P, Q, C]
    prod = (mask * 0.999) * a            # fused scalar_tensor_tensor
    out  = reduce_sum(prod, axis=C)      # segmented reduce
which is bit-exact vs the f32 reference (one-hot sum adds exact zeros).

All loads ride the single SP HWDGE ring (measured faster than splitting
across SP+ACT rings); the output accumulates in SBUF and is stored once.
"""

import numpy as np

import concourse.bacc as bacc
import concourse.mybir as mybir
import concourse.tile as tile
from concourse import bass_utils

B = 2097152  # total batch rows
D = 16       # z feature width
C = 10       # color-counter categories
NCORES = 8
R = B // NCORES   # rows per core = 262144
P = 128           # SBUF partitions
J = R // P        # rows per partition = 2048

_cache: dict[tuple, "bacc.Bacc"] = {}

# Tunables (overridable for A/B benchmarking).
DEFAULTS = dict(
    tile_sizes=(128, 128, 128, 128, 512, 512, 512),
    io_bufs=3,
    store_per_tile=False,
    store_engine="sync",
    accum=False,   # False | "sb2sb": fuse a-add via SWDGE SBUF->SBUF accum
    z_ahead=False,  # issue z-load of tile t+1 before a-load of tile t
)

KBIG = 1024.0  # accum trick: f = a + K*(c - z); K*|c-z| >= K >> 1 for c != z


def _build(attr_index: int, tile_sizes=(512,) * 4, io_bufs=2,
           store_per_tile=False, store_engine="sync", accum=False,
           z_ahead=False) -> "bacc.Bacc":
    tile_sizes = tuple(tile_sizes)
    assert sum(tile_sizes) == J

    nc = bacc.Bacc("TRN2", target_bir_lowering=False, debug=False)

    z_d = nc.dram_tensor("z", [R, D], mybir.dt.float32, kind="ExternalInput")
    a_d = nc.dram_tensor("a", [R, C], mybir.dt.float32, kind="ExternalInput")
    o_d = nc.dram_tensor("out", [R], mybir.dt.float32, kind="ExternalOutput")

    # Partition-major row layout: local row r -> (partition r // J, slot r % J).
    z_t = z_d.ap().rearrange("(p j) d -> p j d", p=P)
    a_t = a_d.ap().rearrange("(p j) c -> p j c", p=P)
    o_t = o_d.ap().rearrange("(p j) -> p j", p=P)

    qmax = max(tile_sizes)

    with tile.TileContext(nc) as tc:
        with (
            tc.tile_pool(name="const", bufs=1) as constp,
            tc.tile_pool(name="io", bufs=io_bufs) as iop,
            tc.tile_pool(name="work", bufs=2) as workp,
            tc.tile_pool(name="osb", bufs=2 if store_per_tile else 1) as outp,
        ):
            iota_step = int(KBIG) if accum else 1
            iota_i = constp.tile([P, C], mybir.dt.int32)
            nc.gpsimd.iota(iota_i, pattern=[[iota_step, C]], base=0,
                           channel_multiplier=0)
            iota_f = constp.tile([P, C], mybir.dt.float32)
            nc.vector.tensor_copy(out=iota_f, in_=iota_i)

            out_sb = None
            if not store_per_tile:
                out_sb = outp.tile([P, J], mybir.dt.float32, name="out_all")

            st_eng = nc.scalar if store_engine == "scalar" else nc.sync

            T = len(tile_sizes)
            starts = [sum(tile_sizes[:t]) for t in range(T)]
            z_tiles: list = [None] * T

            def load_z(t):
                q_ = tile_sizes[t]
                sl_ = slice(starts[t], starts[t] + q_)
                zt = iop.tile([P, q_, D], mybir.dt.float32, tag="zt",
                              padded_shape=[P, qmax, D], name=f"z_tile_{t}")
                nc.sync.dma_start(out=zt, in_=z_t[:, sl_, :])
                z_tiles[t] = zt

            if z_ahead:
                load_z(0)

            for t, q in enumerate(tile_sizes):
                sl = slice(starts[t], starts[t] + q)

                if z_ahead:
                    if t + 1 < T:
                        load_z(t + 1)
                else:
                    load_z(t)
                z_tile = z_tiles[t]
                a_tile = iop.tile([P, q, C], mybir.dt.float32, tag="at",
                                  padded_shape=[P, qmax, C])
                nc.sync.dma_start(out=a_tile, in_=a_t[:, sl, :])

                z_b = z_tile[:, :, attr_index : attr_index + 1].broadcast_to(
                    [P, q, C]
                )
                i_b = iota_f.unsqueeze(1).broadcast_to([P, q, C])
                mask = workp.tile([P, q, C], mybir.dt.float32, tag="mask",
                                  padded_shape=[P, qmax, C])

                if accum == "sb2sb":
                    # f = K*iota - K*z, then f += a via SWDGE SBUF->SBUF accum,
                    # then out = 0.999 * min_abs(f) (= 0.999 * a[idx]).
                    nc.vector.scalar_tensor_tensor(
                        out=mask,
                        in0=z_b,
                        scalar=-KBIG,
                        in1=i_b,
                        op0=mybir.AluOpType.mult,
                        op1=mybir.AluOpType.add,
                    )
                    nc.gpsimd.dma_start(
                        out=mask, in_=a_tile, accum_op=mybir.AluOpType.add
                    )
                    red = outp.tile([P, q], mybir.dt.float32, tag="red",
                                    padded_shape=[P, qmax])
                    nc.vector.tensor_reduce(
                        out=red,
                        in_=mask,
                        axis=mybir.AxisListType.X,
                        op=mybir.AluOpType.min,
                        apply_absolute_value=True,
                    )
                    if store_per_tile:
                        sc = outp.tile([P, q], mybir.dt.float32, tag="sc",
                                       padded_shape=[P, qmax])
                        nc.scalar.mul(out=sc, in_=red, mul=0.999)
                        st_eng.dma_start(out=o_t[:, sl], in_=sc)
                    else:
                        nc.scalar.mul(out=out_sb[:, sl], in_=red, mul=0.999)
                else:
                    nc.vector.tensor_tensor(
                        out=mask, in0=z_b, in1=i_b, op=mybir.AluOpType.is_equal
                    )
                    nc.vector.scalar_tensor_tensor(
                        out=mask,
                        in0=mask,
                        scalar=0.999,
                        in1=a_tile,
                        op0=mybir.AluOpType.mult,
                        op1=mybir.AluOpType.mult,
                    )
                    if store_per_tile:
                        red = outp.tile([P, q], mybir.dt.float32, tag="red",
                                        padded_shape=[P, qmax])
                        nc.vector.tensor_reduce(
                            out=red,
                            in_=mask,
                            axis=mybir.AxisListType.X,
                            op=mybir.AluOpType.add,
                        )
                        st_eng.dma_start(out=o_t[:, sl], in_=red)
                    else:
                        nc.vector.tensor_reduce(
                            out=out_sb[:, sl],
                            in_=mask,
                            axis=mybir.AxisListType.X,
                            op=mybir.AluOpType.add,
                        )

            if not store_per_tile:
                st_eng.dma_start(out=o_t, in_=out_sb)

    nc.compile()
    return nc


def get_nc(attr_index: int = 8, **opts) -> "bacc.Bacc":
    cfg = dict(DEFAULTS)
    cfg.update(opts)
    cfg["tile_sizes"] = tuple(cfg["tile_sizes"])
    key = (int(attr_index), tuple(sorted(cfg.items())))
    if key not in _cache:
        _cache[key] = _build(int(attr_index), **cfg)
    return _cache[key]


def run(z, a, attr_index=8, trace: bool = False, **opts):
    """Run on all 8 cores; returns (full_output, BassKernelResults)."""
    nc = get_nc(attr_index, **opts)
    z = np.ascontiguousarray(np.asarray(z, dtype=np.float32))
    a = np.ascontiguousarray(np.asarray(a, dtype=np.float32))
    assert z.shape == (B, D) and a.shape == (B, C), (z.shape, a.shape)
    in_maps = [
        {"z": z[i * R : (i + 1) * R], "a": a[i * R : (i + 1) * R]}
        for i in range(NCORES)
    ]
    res = bass_utils.run_bass_kernel_spmd(
        nc, in_maps, core_ids=list(range(NCORES)), trace=trace
    )
    out = np.concatenate([r["out"].reshape(R) for r in res.results])
    return out, res


def kernel(z, a, attr_index=8, **_unused):
    out, _ = run(z, a, attr_index)
    return out



# revision 2
# speedup vs baseline: 1.0474x; 1.0474x over previous
"""Trainium2 Bass kernel for the FCNN color-counter valuation function.

Computes out[i] = a[i, int(z[i, attr_index])] * 0.999 for i in [0, B).

Strategy: pure data parallel over 8 NeuronCores (batch sharded). Per core,
rows are laid out partition-major ([128 partitions, J rows each]) so every
DMA is a large contiguous-per-partition transfer.

The gather runs per color category on the vector engine:
    prod[:, :, c] = (z[:, :, attr] == c) * a[:, :, c]   # 10x scalar_tensor_tensor
    red           = reduce_add(prod, axis=C)            # exact: one nonzero/row
    out           = red * 0.999                         # on ACT engine
This costs ~20.5 DVE cycles/row vs ~30 for the broadcast-mask scheme (the
mask build and the fused multiply both run at 1x because of the stride-0
broadcast operand; the per-color form keeps every op at 1x but touches each
a element only twice instead of three times). Result is bit-exact vs the
f32 reference.

Loads ride the SP HWDGE ring; the 0.999 scale and per-tile output stores ride
the ACT engine/ring so they overlap the loads. Tile sizes taper at both ends:
small head tiles fill the pipeline quickly, small tail tiles shrink the
compute+store tail after the last load completes.
"""

import numpy as np

import concourse.bacc as bacc
import concourse.mybir as mybir
import concourse.tile as tile
from concourse import bass_utils

B = 2097152  # total batch rows
D = 16       # z feature width
C = 10       # color-counter categories
NCORES = 8
R = B // NCORES   # rows per core = 262144
P = 128           # SBUF partitions
J = R // P        # rows per partition = 2048

_cache: dict[tuple, "bacc.Bacc"] = {}

# Tunables (overridable for A/B benchmarking).
DEFAULTS = dict(
    tile_sizes=(64, 64, 128, 256, 512, 512, 256, 128, 64, 64),
    io_bufs=3,
    scheme="percolor",   # "percolor" | "mask"
    scale_engine="scalar",  # engine for the *0.999 (percolor): "scalar"|"vector"
    store_engine="scalar",  # engine ring for output stores
)


def _build(attr_index: int, tile_sizes, io_bufs=3, scheme="percolor",
           scale_engine="scalar", store_engine="scalar") -> "bacc.Bacc":
    tile_sizes = tuple(tile_sizes)
    assert sum(tile_sizes) == J

    nc = bacc.Bacc("TRN2", target_bir_lowering=False, debug=False)

    z_d = nc.dram_tensor("z", [R, D], mybir.dt.float32, kind="ExternalInput")
    a_d = nc.dram_tensor("a", [R, C], mybir.dt.float32, kind="ExternalInput")
    o_d = nc.dram_tensor("out", [R], mybir.dt.float32, kind="ExternalOutput")

    # Partition-major row layout: local row r -> (partition r // J, slot r % J).
    z_t = z_d.ap().rearrange("(p j) d -> p j d", p=P)
    a_t = a_d.ap().rearrange("(p j) c -> p j c", p=P)
    o_t = o_d.ap().rearrange("(p j) -> p j", p=P)

    qmax = max(tile_sizes)

    with tile.TileContext(nc) as tc:
        with (
            tc.tile_pool(name="const", bufs=1) as constp,
            tc.tile_pool(name="io", bufs=io_bufs) as iop,
            tc.tile_pool(name="work", bufs=1) as workp,
            tc.tile_pool(name="osb", bufs=2) as outp,
        ):
            st_eng = nc.scalar if store_engine == "scalar" else nc.sync

            iota_f = None
            if scheme == "mask":
                iota_i = constp.tile([P, C], mybir.dt.int32)
                nc.gpsimd.iota(iota_i, pattern=[[1, C]], base=0,
                               channel_multiplier=0)
                iota_f = constp.tile([P, C], mybir.dt.float32)
                nc.vector.tensor_copy(out=iota_f, in_=iota_i)

            T = len(tile_sizes)
            starts = [sum(tile_sizes[:t]) for t in range(T)]

            for t, q in enumerate(tile_sizes):
                sl = slice(starts[t], starts[t] + q)

                z_tile = iop.tile([P, q, D], mybir.dt.float32, tag="zt",
                                  padded_shape=[P, qmax, D])
                nc.sync.dma_start(out=z_tile, in_=z_t[:, sl, :])
                a_tile = iop.tile([P, q, C], mybir.dt.float32, tag="at",
                                  padded_shape=[P, qmax, C])
                nc.sync.dma_start(out=a_tile, in_=a_t[:, sl, :])

                prod = workp.tile([P, q, C], mybir.dt.float32, tag="prod",
                                  padded_shape=[P, qmax, C])
                red = outp.tile([P, q], mybir.dt.float32, tag="red",
                                padded_shape=[P, qmax])

                if scheme == "percolor":
                    z8 = z_tile[:, :, attr_index : attr_index + 1]
                    for c in range(C):
                        nc.vector.scalar_tensor_tensor(
                            out=prod[:, :, c : c + 1],
                            in0=z8,
                            scalar=float(c),
                            in1=a_tile[:, :, c : c + 1],
                            op0=mybir.AluOpType.is_equal,
                            op1=mybir.AluOpType.mult,
                        )
                    nc.vector.tensor_reduce(
                        out=red,
                        in_=prod,
                        axis=mybir.AxisListType.X,
                        op=mybir.AluOpType.add,
                    )
                else:
                    z_b = z_tile[:, :, attr_index : attr_index + 1].broadcast_to(
                        [P, q, C]
                    )
                    i_b = iota_f.unsqueeze(1).broadcast_to([P, q, C])
                    nc.vector.tensor_tensor(
                        out=prod, in0=z_b, in1=i_b,
                        op=mybir.AluOpType.is_equal,
                    )
                    nc.vector.scalar_tensor_tensor(
                        out=prod, in0=prod, scalar=1.0, in1=a_tile,
                        op0=mybir.AluOpType.mult, op1=mybir.AluOpType.mult,
                    )
                    nc.vector.tensor_reduce(
                        out=red, in_=prod,
                        axis=mybir.AxisListType.X, op=mybir.AluOpType.add,
                    )

                sc = outp.tile([P, q], mybir.dt.float32, tag="sc",
                               padded_shape=[P, qmax])
                if scale_engine == "scalar":
                    nc.scalar.mul(out=sc, in_=red, mul=0.999)
                else:
                    nc.vector.tensor_scalar_mul(sc, red, 0.999)
                st_eng.dma_start(out=o_t[:, sl], in_=sc)

    nc.compile()
    return nc


def get_nc(attr_index: int = 8, **opts) -> "bacc.Bacc":
    cfg = dict(DEFAULTS)
    cfg.update(opts)
    cfg["tile_sizes"] = tuple(cfg["tile_sizes"])
    key = (int(attr_index), tuple(sorted(cfg.items())))
    if key not in _cache:
        _cache[key] = _build(int(attr_index), **cfg)
    return _cache[key]


def run(z, a, attr_index=8, trace: bool = False, **opts):
    """Run on all 8 cores; returns (full_output, BassKernelResults)."""
    nc = get_nc(attr_index, **opts)
    z = np.ascontiguousarray(np.asarray(z, dtype=np.float32))
    a = np.ascontiguousarray(np.asarray(a, dtype=np.float32))
    assert z.shape == (B, D) and a.shape == (B, C), (z.shape, a.shape)
    in_maps = [
        {"z": z[i * R : (i + 1) * R], "a": a[i * R : (i + 1) * R]}
        for i in range(NCORES)
    ]
    res = bass_utils.run_bass_kernel_spmd(
        nc, in_maps, core_ids=list(range(NCORES)), trace=trace
    )
    out = np.concatenate([r["out"].reshape(R) for r in res.results])
    return out, res


def kernel(z, a, attr_index=8, **_unused):
    out, _ = run(z, a, attr_index)
    return out
